# revision 6
# baseline (speedup 1.0000x reference)
"""MinGRU Trainium2 kernel.

Reference computation (per batch element b, sequence length T, hidden H):
    k  = x @ W_z + b_z                       # [T, H]
    th = x @ W_h + b_h                       # [T, H]
    a  = sigmoid(-k)            (= 1 - z)
    g  = where(th >= 0, th + 0.5, sigmoid(th)) == max(th + 0.5, sigmoid(th))
    b_ = sigmoid(k) * g         (= z * g)
    h[t] = a[t] * h[t-1] + b_[t]             # linear scan along T
Output h  # [B, T, H]

Strategy: data-parallel over batch (B=8 -> 8 NeuronCores). Host transposes
x[b] to [D, T] so both matmuls produce [H, T] tiles directly (contraction dim
D on partitions for both operands; W is already the lhsT layout [D, H]).
Matmuls run in float32r (full-rate fp32 mode of the PE). The recurrence runs
on the Vector engine's TENSOR_TENSOR_SCAN along the free (T) axis:
state = (a * state) - t with t = (a-1)*g = -b_. Host transposes the [H, T]
result back to [T, H].
"""

import numpy as np

B, T, D, H = 8, 4096, 512, 512
N_CORES = 8
MMN = 512                 # matmul free dim (PSUM bank limit for fp32)
TCH = 1024                # PSUM / ACT / u chunk along T
SCH = 2048                # t-stt and scan chunk along T
NT = T // TCH             # 4
NS = T // SCH             # 2
NM = H // 128             # 4 partition tiles of H
NK = D // 128             # 4 contraction tiles

_cache = {}


def _build():
    import concourse.tile as tile
    from concourse import bacc, mybir

    f32 = mybir.dt.float32
    f32r = mybir.dt.float32r
    AF = mybir.ActivationFunctionType
    ALU = mybir.AluOpType

    nc = bacc.Bacc("TRN2", target_bir_lowering=False, debug=False,
                   num_devices=N_CORES)

    xt_d = nc.dram_tensor("xt", [D, T], f32r, kind="ExternalInput").ap()
    wz_d = nc.dram_tensor("wz", [D, H], f32r, kind="ExternalInput").ap()
    wh_d = nc.dram_tensor("wh", [D, H], f32r, kind="ExternalInput").ap()
    nbz_d = nc.dram_tensor("nbz", [128, NM], f32, kind="ExternalInput").ap()
    bh_d = nc.dram_tensor("bh", [128, NM], f32, kind="ExternalInput").ap()
    bh5_d = nc.dram_tensor("bh5", [128, NM], f32, kind="ExternalInput").ap()
    ht_d = nc.dram_tensor("ht", [H, T], f32, kind="ExternalOutput").ap()

    with tile.TileContext(nc) as tc:
        with (
            tc.tile_pool(name="const", bufs=1) as const,
            tc.tile_pool(name="chunks", bufs=4) as chunks,
            tc.tile_pool(name="psum", bufs=2, space="PSUM") as psum,
        ):
            # weights first (needed by the first matmul), then the first
            # T-chunk of x[b].T, then the rest streamed in T-chunks.
            wz_s = [const.tile([128, H], f32r, tag=f"wz{k}", name=f"wz{k}")
                    for k in range(NK)]
            wh_s = [const.tile([128, H], f32r, tag=f"wh{k}", name=f"wh{k}")
                    for k in range(NK)]
            xt_s = [const.tile([128, T], f32r, tag=f"xt{k}", name=f"xt{k}")
                    for k in range(NK)]
            for k in range(NK):
                nc.sync.dma_start(wz_s[k][:], wz_d[k * 128:(k + 1) * 128, :])
            for k in range(NK):
                nc.sync.dma_start(xt_s[k][:, 0:TCH], xt_d[k * 128:(k + 1) * 128, 0:TCH])
            nbz_s = const.tile([128, NM], f32, tag="nbz")
            nc.sync.dma_start(nbz_s[:], nbz_d[:])
            bh_s = const.tile([128, NM], f32, tag="bh")
            nc.sync.dma_start(bh_s[:], bh_d[:])
            bh5_s = const.tile([128, NM], f32, tag="bh5")
            nc.sync.dma_start(bh5_s[:], bh5_d[:])
            for k in range(NK):
                nc.sync.dma_start(wh_s[k][:], wh_d[k * 128:(k + 1) * 128, :])
            for tc_i in range(1, NT):
                tsl = slice(tc_i * TCH, (tc_i + 1) * TCH)
                for k in range(NK):
                    nc.sync.dma_start(xt_s[k][:, tsl], xt_d[k * 128:(k + 1) * 128, tsl])

            for m in range(NM):
                msl = slice(m * 128, (m + 1) * 128)
                a_half = [None, None]   # [128, SCH] tiles per half
                u_half = [None, None]
                h_prev = None
                for tc_i in range(NT):
                    tsl = slice(tc_i * TCH, (tc_i + 1) * TCH)
                    half = tc_i % 2     # position inside the SCH window
                    if half == 0:
                        a_half[0] = chunks.tile([128, SCH], f32, tag="a",
                                                name="a", bufs=2)
                        u_half[0] = chunks.tile([128, SCH], f32, tag="u",
                                                name="u", bufs=2)
                    a = a_half[0]
                    u = u_half[0]
                    csl = slice(half * TCH, (half + 1) * TCH)
                    psK = psum.tile([128, TCH], f32, tag="psK")
                    psT = psum.tile([128, TCH], f32, tag="psT")
                    for sub in range(TCH // MMN):
                        nsl = slice(tc_i * TCH + sub * MMN,
                                    tc_i * TCH + (sub + 1) * MMN)
                        osl = slice(sub * MMN, (sub + 1) * MMN)
                        for k in range(NK):
                            nc.tensor.matmul(psK[:, osl], wz_s[k][:, msl],
                                             xt_s[k][:, nsl],
                                             start=(k == 0), stop=(k == NK - 1))
                        for k in range(NK):
                            nc.tensor.matmul(psT[:, osl], wh_s[k][:, msl],
                                             xt_s[k][:, nsl],
                                             start=(k == 0), stop=(k == NK - 1))
                    # a = sigmoid(-(k0 + b_z))
                    nc.scalar.activation(a[:, csl], psK[:], AF.Sigmoid,
                                         bias=nbz_s[:, m:m + 1], scale=-1.0)
                    # sg = sigmoid(th0 + b_h)
                    sg = chunks.tile([128, TCH], f32, tag="sg", bufs=3)
                    nc.scalar.activation(sg[:], psT[:], AF.Sigmoid,
                                         bias=bh_s[:, m:m + 1], scale=1.0)
                    # u = max(th0 + (b_h + 0.5), sg)
                    nc.vector.scalar_tensor_tensor(
                        u[:, csl], psT[:], bh5_s[:, m:m + 1], sg[:],
                        ALU.add, ALU.max)
                    if half == 1:
                        # t = (a - 1) * u  (= -b_)
                        tt = chunks.tile([128, SCH], f32, tag="tt", bufs=2)
                        nc.vector.scalar_tensor_tensor(
                            tt[:], a[:], 1.0, u[:], ALU.subtract, ALU.mult)
                        # h[t] = a[t]*h[t-1] - t[t]
                        h = chunks.tile([128, SCH], f32, tag="h", bufs=2)
                        init = (0.0 if h_prev is None
                                else h_prev[:, SCH - 1:SCH])
                        nc.vector.tensor_tensor_scan(h[:], a[:], tt[:], init,
                                                     ALU.mult, ALU.subtract)
                        h_prev = h
                        ssl = slice((tc_i - 1) * TCH, (tc_i + 1) * TCH)
                        nc.sync.dma_start(ht_d[msl, ssl], h[:])

    nc.compile()
    return nc


def kernel(x, W_z, b_z, W_h, b_h):
    from concourse.bass_utils import run_bass_kernel_spmd

    if "nc" not in _cache:
        _cache["nc"] = _build()
    nc = _cache["nc"]

    x = np.asarray(x, dtype=np.float32)
    W_z = np.ascontiguousarray(np.asarray(W_z, dtype=np.float32))
    W_h = np.ascontiguousarray(np.asarray(W_h, dtype=np.float32))
    b_z = np.asarray(b_z, dtype=np.float32)
    b_h = np.asarray(b_h, dtype=np.float32)

    nbz = np.ascontiguousarray((-b_z).reshape(NM, 128).T)
    bh = np.ascontiguousarray(b_h.reshape(NM, 128).T)
    bh5 = np.ascontiguousarray((b_h + 0.5).reshape(NM, 128).T)

    in_maps = []
    for b in range(B):
        in_maps.append({
            "xt": np.ascontiguousarray(x[b].T),
            "wz": W_z,
            "wh": W_h,
            "nbz": nbz,
            "bh": bh,
            "bh5": bh5,
        })

    import os
    kwargs = {}
    if os.environ.get("KERNEL_TRACE"):
        kwargs = dict(trace=True, tmpdir=os.environ.get("KERNEL_TMPDIR"))
    res = run_bass_kernel_spmd(nc, in_maps, core_ids=list(range(N_CORES)),
                               **kwargs)
    _cache["last_results"] = res

    out = np.empty((B, T, H), dtype=np.float32)
    for b in range(B):
        out[b] = res.results[b]["ht"].T
    return out


# revision 7
# speedup vs baseline: 1.0102x; 1.0102x over previous
"""MinGRU Trainium2 kernel.

Reference computation (per batch element b, sequence length T, hidden H):
    k  = x @ W_z + b_z                       # [T, H]
    th = x @ W_h + b_h                       # [T, H]
    a  = sigmoid(-k)            (= 1 - z)
    g  = where(th >= 0, th + 0.5, sigmoid(th)) == max(th + 0.5, sigmoid(th))
    b_ = sigmoid(k) * g         (= z * g)
    h[t] = a[t] * h[t-1] + b_[t]             # linear scan along T
Output h  # [B, T, H]

Strategy: data-parallel over batch (B=8 -> 8 NeuronCores). Host transposes
x[b] to [D, T] so both matmuls produce [H, T] tiles directly (contraction dim
D on partitions for both operands; W is already the lhsT layout [D, H]).
Matmuls run in float32r (full-rate fp32 mode of the PE). The recurrence runs
on the Vector engine's TENSOR_TENSOR_SCAN along the free (T) axis:
state = (a * state) - t with t = (a-1)*g = -b_. Host transposes the [H, T]
result back to [T, H].
"""

import numpy as np

B, T, D, H = 8, 4096, 512, 512
N_CORES = 8
MMN = 512                 # matmul free dim (PSUM bank limit for fp32)
TCH = 1024                # PSUM / ACT / u chunk along T
SCH = 2048                # t-stt and scan chunk along T
NT = T // TCH             # 4
NS = T // SCH             # 2
NM = H // 128             # 4 partition tiles of H
NK = D // 128             # 4 contraction tiles

_cache = {}


def _build():
    import concourse.tile as tile
    from concourse import bacc, mybir

    f32 = mybir.dt.float32
    f32r = mybir.dt.float32r
    AF = mybir.ActivationFunctionType
    ALU = mybir.AluOpType

    nc = bacc.Bacc("TRN2", target_bir_lowering=False, debug=False,
                   num_devices=N_CORES)

    xt_d = nc.dram_tensor("xt", [D, T], f32r, kind="ExternalInput").ap()
    wz_d = nc.dram_tensor("wz", [D, H], f32r, kind="ExternalInput").ap()
    wh_d = nc.dram_tensor("wh", [D, H], f32r, kind="ExternalInput").ap()
    nbz_d = nc.dram_tensor("nbz", [128, NM], f32, kind="ExternalInput").ap()
    bh_d = nc.dram_tensor("bh", [128, NM], f32, kind="ExternalInput").ap()
    bh5_d = nc.dram_tensor("bh5", [128, NM], f32, kind="ExternalInput").ap()
    ht_d = nc.dram_tensor("ht", [H, T], f32, kind="ExternalOutput").ap()

    with tile.TileContext(nc) as tc:
        with (
            tc.tile_pool(name="const", bufs=1) as const,
            tc.tile_pool(name="chunks", bufs=4) as chunks,
            tc.tile_pool(name="psum", bufs=2, space="PSUM") as psum,
        ):
            # weights first (needed by the first matmul), then the first
            # T-chunk of x[b].T, then the rest streamed in T-chunks.
            wz_s = [const.tile([128, H], f32r, tag=f"wz{k}", name=f"wz{k}")
                    for k in range(NK)]
            wh_s = [const.tile([128, H], f32r, tag=f"wh{k}", name=f"wh{k}")
                    for k in range(NK)]
            xt_s = [const.tile([128, T], f32r, tag=f"xt{k}", name=f"xt{k}")
                    for k in range(NK)]
            for k in range(NK):
                nc.scalar.dma_start(wz_s[k][:], wz_d[k * 128:(k + 1) * 128, :])
            for k in range(NK):
                nc.sync.dma_start(xt_s[k][:, 0:TCH], xt_d[k * 128:(k + 1) * 128, 0:TCH])
            nbz_s = const.tile([128, NM], f32, tag="nbz")
            nc.scalar.dma_start(nbz_s[:], nbz_d[:])
            bh_s = const.tile([128, NM], f32, tag="bh")
            nc.scalar.dma_start(bh_s[:], bh_d[:])
            bh5_s = const.tile([128, NM], f32, tag="bh5")
            nc.scalar.dma_start(bh5_s[:], bh5_d[:])
            for k in range(NK):
                nc.scalar.dma_start(wh_s[k][:], wh_d[k * 128:(k + 1) * 128, :])
            for tc_i in range(1, NT):
                tsl = slice(tc_i * TCH, (tc_i + 1) * TCH)
                for k in range(NK):
                    nc.sync.dma_start(xt_s[k][:, tsl], xt_d[k * 128:(k + 1) * 128, tsl])

            for m in range(NM):
                msl = slice(m * 128, (m + 1) * 128)
                a_half = [None, None]   # [128, SCH] tiles per half
                u_half = [None, None]
                h_prev = None
                for tc_i in range(NT):
                    tsl = slice(tc_i * TCH, (tc_i + 1) * TCH)
                    half = tc_i % 2     # position inside the SCH window
                    if half == 0:
                        a_half[0] = chunks.tile([128, SCH], f32, tag="a",
                                                name="a", bufs=2)
                        u_half[0] = chunks.tile([128, SCH], f32, tag="u",
                                                name="u", bufs=2)
                    a = a_half[0]
                    u = u_half[0]
                    csl = slice(half * TCH, (half + 1) * TCH)
                    psK = psum.tile([128, TCH], f32, tag="psK")
                    psT = psum.tile([128, TCH], f32, tag="psT")
                    for sub in range(TCH // MMN):
                        nsl = slice(tc_i * TCH + sub * MMN,
                                    tc_i * TCH + (sub + 1) * MMN)
                        osl = slice(sub * MMN, (sub + 1) * MMN)
                        for k in range(NK):
                            nc.tensor.matmul(psK[:, osl], wz_s[k][:, msl],
                                             xt_s[k][:, nsl],
                                             start=(k == 0), stop=(k == NK - 1))
                        for k in range(NK):
                            nc.tensor.matmul(psT[:, osl], wh_s[k][:, msl],
                                             xt_s[k][:, nsl],
                                             start=(k == 0), stop=(k == NK - 1))
                    # a = sigmoid(-(k0 + b_z))
                    nc.scalar.activation(a[:, csl], psK[:], AF.Sigmoid,
                                         bias=nbz_s[:, m:m + 1], scale=-1.0)
                    # sg = sigmoid(th0 + b_h)
                    sg = chunks.tile([128, TCH], f32, tag="sg", bufs=3)
                    nc.scalar.activation(sg[:], psT[:], AF.Sigmoid,
                                         bias=bh_s[:, m:m + 1], scale=1.0)
                    # l = th0 + (b_h + 0.5)
                    l = chunks.tile([128, TCH], f32, tag="l", bufs=3)
                    nc.scalar.activation(l[:], psT[:], AF.Identity,
                                         bias=bh5_s[:, m:m + 1], scale=1.0)
                    # u = max(l, sg)
                    nc.vector.tensor_tensor(u[:, csl], l[:], sg[:], ALU.max)
                    if half == 1:
                        # t = (a - 1) * u  (= -b_)
                        tt = chunks.tile([128, SCH], f32, tag="tt", bufs=2)
                        nc.vector.scalar_tensor_tensor(
                            tt[:], a[:], 1.0, u[:], ALU.subtract, ALU.mult)
                        # h[t] = a[t]*h[t-1] - t[t]
                        h = chunks.tile([128, SCH], f32, tag="h", bufs=2)
                        init = (0.0 if h_prev is None
                                else h_prev[:, SCH - 1:SCH])
                        nc.vector.tensor_tensor_scan(h[:], a[:], tt[:], init,
                                                     ALU.mult, ALU.subtract)
                        h_prev = h
                        ssl = slice((tc_i - 1) * TCH, (tc_i + 1) * TCH)
                        nc.sync.dma_start(ht_d[msl, ssl], h[:])

    nc.compile()
    return nc


def kernel(x, W_z, b_z, W_h, b_h):
    from concourse.bass_utils import run_bass_kernel_spmd

    if "nc" not in _cache:
        _cache["nc"] = _build()
    nc = _cache["nc"]

    x = np.asarray(x, dtype=np.float32)
    W_z = np.ascontiguousarray(np.asarray(W_z, dtype=np.float32))
    W_h = np.ascontiguousarray(np.asarray(W_h, dtype=np.float32))
    b_z = np.asarray(b_z, dtype=np.float32)
    b_h = np.asarray(b_h, dtype=np.float32)

    nbz = np.ascontiguousarray((-b_z).reshape(NM, 128).T)
    bh = np.ascontiguousarray(b_h.reshape(NM, 128).T)
    bh5 = np.ascontiguousarray((b_h + 0.5).reshape(NM, 128).T)

    in_maps = []
    for b in range(B):
        in_maps.append({
            "xt": np.ascontiguousarray(x[b].T),
            "wz": W_z,
            "wh": W_h,
            "nbz": nbz,
            "bh": bh,
            "bh5": bh5,
        })

    import os
    kwargs = {}
    if os.environ.get("KERNEL_TRACE"):
        kwargs = dict(trace=True, tmpdir=os.environ.get("KERNEL_TMPDIR"))
    res = run_bass_kernel_spmd(nc, in_maps, core_ids=list(range(N_CORES)),
                               **kwargs)
    _cache["last_results"] = res

    out = np.empty((B, T, H), dtype=np.float32)
    for b in range(B):
        out[b] = res.results[b]["ht"].T
    return out
